# revision 6
# baseline (speedup 1.0000x reference)
"""Single-head causal attention on 8 Trainium2 NeuronCores (Bass/Tile).

Problem: x[4, 2048, 1024], Wq/Wk/Wv[1024, 1024] fp32 ->
         softmax(causal(q k^T / sqrt(1024))) v,  q/k/v = x @ W.

Sharding (uniform SPMD, one NEFF for all 8 cores):
  - 2 cores per batch. Each core computes the full K/V projection for its
    batch (recompute instead of collectives) plus attention for a balanced,
    interleaved quarter of all query rows (1024 rows/core).
  - Query 256-row groups 0..7 of the sequence are split between the batch
    pair as {7,4,3,0} (even core) / {6,5,2,1} (odd core): both sets need
    exactly the same number of causal key tiles per slot position, so the
    compiled kernel is identical across cores; only input data differs.
  - Per-slot key-tile bounds are the uniform elementwise max [16,12,8,4];
    the last 4 key tiles of each slot get a host-computed 0/1 mask input
    (causal + zero padding), everything earlier is causally full.

Kernel structure per core:
  Phase A: transpose x on-chip (PE transpose) -> xT; project kT = Wk^T x^T
           ([e,s] layout) and v = x Wv ([s,e]); spill both to DRAM scratch.
  Phase B: same for the core's own 1024 query rows -> qT resident in SBUF.
  Phase C: flash-style attention, key-block (512) outer loop:
           scoresT[k,q] = kT^T qT (PSUM, fp32 accum), exp on ACT engine
           (scale=1/32 folded in), causal mask via 0/1 multiply, then
           O += w^T V and row-sums l += w^T 1 on the PE; final out = O/l.

All matmuls run as float32r (TF32-class fast fp32 path, 1 cyc/row at
moving dim >= 256) via bitcast; PSUM accumulation is fp32.
"""

import numpy as np

B, S, D = 4, 2048, 1024
P = 128
QL = 1024  # query rows per core
GW = 256  # query group width
NSLOT = 4  # query groups per core
BOUNDS = (16, 12, 8, 4)  # uniform per-slot key-tile bounds
GROUPS = {0: (7, 4, 3, 0), 1: (6, 5, 2, 1)}  # 256-row q-groups per parity
NMASK = 4  # masked key tiles per slot (the last 4)
SCALE = 1.0 / np.sqrt(np.float32(D))

_cached = {}


def _build_bass():
    import concourse.bacc as bacc
    import concourse.mybir as mybir
    import concourse.tile as tile
    from concourse.masks import make_identity
    from contextlib import ExitStack

    f32 = mybir.dt.float32
    f32r = mybir.dt.float32r

    nc = bacc.Bacc("TRN2")
    x_d = nc.declare_dram_parameter("x", [S, D], f32r, isOutput=False)
    xq_d = nc.declare_dram_parameter("xq", [QL, D], f32r, isOutput=False)
    wq_d = nc.declare_dram_parameter("Wq", [D, D], f32r, isOutput=False)
    wk_d = nc.declare_dram_parameter("Wk", [D, D], f32r, isOutput=False)
    wv_d = nc.declare_dram_parameter("Wv", [D, D], f32r, isOutput=False)
    masks_d = nc.declare_dram_parameter(
        "masks", [NSLOT, NMASK, P, GW], f32r, isOutput=False
    )
    out_d = nc.declare_dram_parameter("out", [QL, D], f32, isOutput=True)

    kT_dram = nc.dram_tensor("kT_scratch", [D, S], f32r)  # [e, s]
    v_dram = nc.dram_tensor("v_scratch", [S, D], f32r)  # [s, e]

    DT = D // P  # 8 d-tiles
    ET = D // P  # 8 e-tiles
    SB = 512  # s-block for projections
    KB = 512  # key block in attention
    NKB = S // KB  # 4 key blocks

    with tile.TileContext(nc, pool_alloc_mode="queue") as tc, ExitStack() as top:
        cpool = top.enter_context(tc.tile_pool(name="const", bufs=1))
        ident_f = cpool.tile([P, P], f32)
        make_identity(nc, ident_f)
        ident = cpool.tile([P, P], f32r)
        nc.vector.tensor_copy(ident, ident_f)
        ones_f = cpool.tile([P, 2], f32)
        nc.gpsimd.memset(ones_f, 1.0)
        ones = cpool.tile([P, 2], f32r)
        nc.vector.tensor_copy(ones, ones_f)

        # qT stays resident from phase B through phase C: [e, q] layout.
        qT_pool = top.enter_context(tc.tile_pool(name="qT", bufs=1))
        qT = [qT_pool.tile([P, QL], f32r, name=f"qT{e}") for e in range(ET)]

        def load_xn(xn_pool, src_dram, row0):
            xns = []
            for st in range(4):
                xn = xn_pool.tile([P, D], f32r, tag="xn", name=f"xn{st}")
                nc.sync.dma_start(xn, src_dram[row0 + st * P : row0 + (st + 1) * P, :])
                xns.append(xn)
            return xns

        def do_transposes(tpsum, xns, xT_tiles):
            for dt in range(DT):
                for st in range(4):
                    tp = tpsum.tile([P, P], f32r, tag="tp", name="tp")
                    nc.tensor.transpose(tp, xns[st][:, dt * P : (dt + 1) * P], ident)
                    nc.vector.tensor_copy(xT_tiles[dt][:, st * P : (st + 1) * P], tp)

        # ---------------- Phase A: kT / v projection of full x ----------------
        with ExitStack() as pa:
            xn_pool = pa.enter_context(tc.tile_pool(name="xnA", bufs=8))
            xT_pool = pa.enter_context(tc.tile_pool(name="xTA", bufs=2))
            tpsum = pa.enter_context(tc.tile_pool(name="tpsA", bufs=4, space="PSUM"))
            ppool = pa.enter_context(tc.tile_pool(name="ppsA", bufs=4, space="PSUM"))
            stage = pa.enter_context(tc.tile_pool(name="stgA", bufs=6))

            xns_next = load_xn(xn_pool, x_d, 0)

            wkv_pool = pa.enter_context(tc.tile_pool(name="wkv", bufs=1))
            wk_sb = [wkv_pool.tile([P, D], f32r, name=f"wk{d}") for d in range(DT)]
            wv_sb = [wkv_pool.tile([P, D], f32r, name=f"wv{d}") for d in range(DT)]
            for d in range(DT):
                nc.sync.dma_start(wk_sb[d], wk_d[d * P : (d + 1) * P, :])
            for d in range(DT):
                nc.sync.dma_start(wv_sb[d], wv_d[d * P : (d + 1) * P, :])

            for sb in range(S // SB):
                xns = xns_next
                if sb + 1 < S // SB:
                    xns_next = load_xn(xn_pool, x_d, (sb + 1) * SB)
                xT = [
                    xT_pool.tile([P, SB], f32r, tag=f"xT{dt}", name=f"xT{dt}")
                    for dt in range(DT)
                ]
                do_transposes(tpsum, xns, xT)

                # kT[e, s] = sum_d Wk[d,e]^T x^T[d,s]
                for et in range(ET):
                    pk = ppool.tile([P, SB], f32, tag="pp", name="pk")
                    for dt in range(DT):
                        nc.tensor.matmul(
                            pk,
                            lhsT=wk_sb[dt][:, et * P : (et + 1) * P],
                            rhs=xT[dt],
                            start=(dt == 0),
                            stop=(dt == DT - 1),
                        )
                    ks = stage.tile([P, SB], f32r, tag="stg", name="ks")
                    nc.scalar.copy(ks, pk)
                    nc.sync.dma_start(
                        kT_dram[et * P : (et + 1) * P, sb * SB : (sb + 1) * SB], ks
                    )
                # v[s, e] = sum_d (x^T[d,s])^T Wv[d,e]
                for st in range(4):
                    for eh in range(2):
                        pv = ppool.tile([P, SB], f32, tag="pp", name="pv")
                        for dt in range(DT):
                            nc.tensor.matmul(
                                pv,
                                lhsT=xT[dt][:, st * P : (st + 1) * P],
                                rhs=wv_sb[dt][:, eh * 512 : (eh + 1) * 512],
                                start=(dt == 0),
                                stop=(dt == DT - 1),
                            )
                        vs = stage.tile([P, SB], f32r, tag="stg", name="vs")
                        nc.scalar.copy(vs, pv)
                        nc.sync.dma_start(
                            v_dram[
                                sb * SB + st * P : sb * SB + (st + 1) * P,
                                eh * 512 : (eh + 1) * 512,
                            ],
                            vs,
                        )

        # ---------------- Phase B: qT projection of own query rows ----------------
        with ExitStack() as pb:
            wq_pool = pb.enter_context(tc.tile_pool(name="wq", bufs=1))
            wq_sb = [wq_pool.tile([P, D], f32r, name=f"wq{d}") for d in range(DT)]
            for d in range(DT):
                nc.sync.dma_start(wq_sb[d], wq_d[d * P : (d + 1) * P, :])

            xn_pool = pb.enter_context(tc.tile_pool(name="xnB", bufs=8))
            xT_pool = pb.enter_context(tc.tile_pool(name="xTB", bufs=2))
            tpsum = pb.enter_context(tc.tile_pool(name="tpsB", bufs=4, space="PSUM"))
            ppool = pb.enter_context(tc.tile_pool(name="ppsB", bufs=4, space="PSUM"))

            xns_next = load_xn(xn_pool, xq_d, 0)
            for sb in range(QL // SB):
                xns = xns_next
                if sb + 1 < QL // SB:
                    xns_next = load_xn(xn_pool, xq_d, (sb + 1) * SB)
                xT = [
                    xT_pool.tile([P, SB], f32r, tag=f"xTq{dt}", name=f"xTq{dt}")
                    for dt in range(DT)
                ]
                do_transposes(tpsum, xns, xT)
                for et in range(ET):
                    pq = ppool.tile([P, SB], f32, tag="ppq", name="pq")
                    for dt in range(DT):
                        nc.tensor.matmul(
                            pq,
                            lhsT=wq_sb[dt][:, et * P : (et + 1) * P],
                            rhs=xT[dt],
                            start=(dt == 0),
                            stop=(dt == DT - 1),
                        )
                    nc.scalar.copy(qT[et][:, sb * SB : (sb + 1) * SB], pq)

        # ---------------- Phase C: attention ----------------
        with ExitStack() as pc:
            mpool = pc.enter_context(tc.tile_pool(name="masks", bufs=1))
            masks_sb = mpool.tile([P, NSLOT * NMASK * GW], f32r)
            for j in range(NSLOT):
                for m in range(NMASK):
                    col = (j * NMASK + m) * GW
                    nc.sync.dma_start(masks_sb[:, col : col + GW], masks_d[j, m])

            acc_pool = pc.enter_context(tc.tile_pool(name="acc", bufs=1))
            # O accumulators: per (slot, q-half) a [128, 1024] fp32 tile.
            O_sb = [
                [acc_pool.tile([P, D], f32, name=f"O{j}_{h}") for h in range(2)]
                for j in range(NSLOT)
            ]
            l_sb = acc_pool.tile([P, 2 * NSLOT], f32)  # row sums, col = 2*slot+half

            kv_pool = pc.enter_context(tc.tile_pool(name="kv", bufs=2))
            w_pool = pc.enter_context(tc.tile_pool(name="wT", bufs=6))
            fin_pool = pc.enter_context(tc.tile_pool(name="fin", bufs=4))
            rec_pool = pc.enter_context(tc.tile_pool(name="recp", bufs=4))
            spsum = pc.enter_context(tc.tile_pool(name="sps", bufs=2, space="PSUM"))
            opsum = pc.enter_context(tc.tile_pool(name="ops", bufs=4, space="PSUM"))
            lpsum = pc.enter_context(tc.tile_pool(name="lps", bufs=2, space="PSUM"))

            for kb in range(NKB):
                kT_sb = [
                    kv_pool.tile([P, KB], f32r, tag=f"kT{e}", name=f"kTs{e}")
                    for e in range(ET)
                ]
                for e in range(ET):
                    nc.sync.dma_start(
                        kT_sb[e], kT_dram[e * P : (e + 1) * P, kb * KB : (kb + 1) * KB]
                    )
                v_sb = [
                    kv_pool.tile([P, D], f32r, tag=f"v{t}", name=f"vs{t}")
                    for t in range(4)
                ]
                for t in range(4):
                    nc.sync.dma_start(
                        v_sb[t],
                        v_dram[kb * KB + t * P : kb * KB + (t + 1) * P, :],
                    )

                for j in range(NSLOT):
                    if kb * 4 >= BOUNDS[j]:
                        continue
                    qcol = j * GW
                    wts = []
                    for kt in range(4):
                        K = kb * 4 + kt  # global key tile
                        sp = spsum.tile([P, GW], f32, tag="sp", name="sp")
                        for e in range(ET):
                            nc.tensor.matmul(
                                sp,
                                lhsT=kT_sb[e][:, kt * P : (kt + 1) * P],
                                rhs=qT[e][:, qcol : qcol + GW],
                                start=(e == 0),
                                stop=(e == ET - 1),
                            )
                        wt = w_pool.tile([P, GW], f32r, tag="wt", name="wt")
                        # w = exp(scores / sqrt(d_k)); scale folded into ACT.
                        nc.scalar.activation(
                            wt, sp, mybir.ActivationFunctionType.Exp, scale=float(SCALE)
                        )
                        m = K - (BOUNDS[j] - NMASK)
                        if m >= 0:
                            mcol = (j * NMASK + m) * GW
                            nc.vector.tensor_mul(
                                wt, wt, masks_sb[:, mcol : mcol + GW]
                            )
                        wts.append(wt)

                    lp = lpsum.tile([P, 4], f32, tag="lp", name="lp")
                    for h in range(2):
                        for eh in range(2):
                            op = opsum.tile([P, 512], f32, tag="op", name="op")
                            for kt in range(4):
                                nc.tensor.matmul(
                                    op,
                                    lhsT=wts[kt][:, h * P : (h + 1) * P],
                                    rhs=v_sb[kt][:, eh * 512 : (eh + 1) * 512],
                                    start=(kt == 0),
                                    stop=(kt == 3),
                                )
                            dst = O_sb[j][h][:, eh * 512 : (eh + 1) * 512]
                            if kb == 0:
                                nc.vector.tensor_copy(dst, op)
                            else:
                                nc.vector.tensor_add(dst, dst, op)
                        for kt in range(4):
                            nc.tensor.matmul(
                                lp[:, 2 * h : 2 * h + 2],
                                lhsT=wts[kt][:, h * P : (h + 1) * P],
                                rhs=ones,
                                start=(kt == 0),
                                stop=(kt == 3),
                            )
                    for h in range(2):
                        lcol = l_sb[:, 2 * j + h : 2 * j + h + 1]
                        if kb == 0:
                            nc.vector.tensor_copy(lcol, lp[:, 2 * h : 2 * h + 1])
                        else:
                            nc.vector.tensor_add(lcol, lcol, lp[:, 2 * h : 2 * h + 1])

                    if kb == BOUNDS[j] // 4 - 1:
                        # last key block for this slot: normalize + store now
                        recip = rec_pool.tile([P, 2], f32, tag="rc", name="recip")
                        nc.vector.reciprocal(recip, l_sb[:, 2 * j : 2 * j + 2])
                        for h in range(2):
                            o = fin_pool.tile([P, D], f32, tag="fo", name="fo")
                            nc.scalar.activation(
                                o,
                                O_sb[j][h],
                                mybir.ActivationFunctionType.Copy,
                                scale=recip[:, h : h + 1],
                            )
                            row = j * GW + h * P
                            nc.sync.dma_start(out_d[row : row + P, :], o)

    nc.compile()
    return nc


def _host_inputs(x, Wq, Wk, Wv):
    in_maps = []
    for c in range(8):
        b, par = c // 2, c % 2
        groups = GROUPS[par]
        rows = np.concatenate(
            [np.arange(GW * g, GW * g + GW) for g in groups]
        )
        xq = np.ascontiguousarray(x[b][rows])
        masks = np.zeros((NSLOT, NMASK, P, GW), np.float32)
        for j, g in enumerate(groups):
            bj = BOUNDS[j]
            for m, kt in enumerate(range(bj - NMASK, bj)):
                kg = P * kt + np.arange(P)[:, None]
                qg = GW * g + np.arange(GW)[None, :]
                masks[j, m] = (kg <= qg).astype(np.float32)
        in_maps.append(
            {
                "x": np.ascontiguousarray(x[b]),
                "xq": xq,
                "Wq": Wq,
                "Wk": Wk,
                "Wv": Wv,
                "masks": masks,
            }
        )
    return in_maps


def kernel(x, Wq, Wk, Wv):
    from concourse.bass_utils import run_bass_kernel_spmd

    x = np.asarray(x, dtype=np.float32)
    Wq = np.ascontiguousarray(np.asarray(Wq, dtype=np.float32))
    Wk = np.ascontiguousarray(np.asarray(Wk, dtype=np.float32))
    Wv = np.ascontiguousarray(np.asarray(Wv, dtype=np.float32))

    if "nc" not in _cached:
        _cached["nc"] = _build_bass()
    nc = _cached["nc"]

    in_maps = _host_inputs(x, Wq, Wk, Wv)
    res = run_bass_kernel_spmd(nc, in_maps, core_ids=list(range(8)))
    _cached["last_result"] = res

    out = np.zeros((B, S, D), np.float32)
    for c in range(8):
        b, par = c // 2, c % 2
        oc = res.results[c]["out"]
        for j, g in enumerate(GROUPS[par]):
            out[b, GW * g : GW * g + GW] = oc[GW * j : GW * j + GW]
    return out


# revision 7
# speedup vs baseline: 2.2907x; 2.2907x over previous
"""Single-head causal attention on 8 Trainium2 NeuronCores (Bass/Tile).

Problem: x[4, 2048, 1024], Wq/Wk/Wv[1024, 1024] fp32 ->
         softmax(causal(q k^T / sqrt(1024))) v,  q/k/v = x @ W.

Sharding (uniform SPMD, one NEFF for all 8 cores):
  - 2 cores per batch. Each core computes the full K/V projection for its
    batch (recompute instead of collectives) plus attention for a balanced,
    interleaved quarter of all query rows (1024 rows/core).
  - Query 256-row groups 0..7 of the sequence are split between the batch
    pair as {7,4,3,0} (even core) / {6,5,2,1} (odd core): both sets need
    exactly the same number of causal key tiles per slot position, so the
    compiled kernel is identical across cores; only input data differs.
  - Per-slot key-tile bounds are the uniform elementwise max [16,12,8,4];
    the last 4 key tiles of each slot get a host-computed 0/1 mask input
    (causal + zero padding), everything earlier is causally full.

Kernel structure per core:
  Phase A: transpose x on-chip (PE transpose) -> xT; project kT = Wk^T x^T
           ([e,s] layout) and v = x Wv ([s,e]); spill both to DRAM scratch.
  Phase B: same for the core's own 1024 query rows -> qT resident in SBUF.
  Phase C: flash-style attention, key-block (512) outer loop:
           scoresT[k,q] = kT^T qT (PSUM, fp32 accum), exp on ACT engine
           (scale=1/32 folded in), causal mask via 0/1 multiply, then
           O += w^T V and row-sums l += w^T 1 on the PE; final out = O/l.

All matmuls run as float32r (TF32-class fast fp32 path, 1 cyc/row at
moving dim >= 256) via bitcast; PSUM accumulation is fp32.
"""

import numpy as np

B, S, D = 4, 2048, 1024
P = 128
QL = 1024  # query rows per core
GW = 256  # query group width
NSLOT = 4  # query groups per core
BOUNDS = (16, 12, 8, 4)  # uniform per-slot key-tile bounds
GROUPS = {0: (7, 4, 3, 0), 1: (6, 5, 2, 1)}  # 256-row q-groups per parity
NMASK = 4  # masked key tiles per slot (the last 4)
SCALE = 1.0 / np.sqrt(np.float32(D))

_cached = {}


def _build_bass():
    import concourse.bacc as bacc
    import concourse.mybir as mybir
    import concourse.tile as tile
    from concourse.masks import make_identity
    from contextlib import ExitStack

    f32 = mybir.dt.float32
    f32r = mybir.dt.float32r

    nc = bacc.Bacc("TRN2")
    xk_d = nc.declare_dram_parameter("xk", [QL, D], f32r, isOutput=False)
    xq_d = nc.declare_dram_parameter("xq", [QL, D], f32r, isOutput=False)
    wq_d = nc.declare_dram_parameter("Wq", [D, D], f32r, isOutput=False)
    wk_d = nc.declare_dram_parameter("Wk", [D, D], f32r, isOutput=False)
    wv_d = nc.declare_dram_parameter("Wv", [D, D], f32r, isOutput=False)
    masks_d = nc.declare_dram_parameter(
        "masks", [NSLOT, NMASK, P, GW], f32r, isOutput=False
    )
    out_d = nc.declare_dram_parameter("out", [QL, D], f32, isOutput=True)

    # kv_send[0] = own kT half [e, s_local]; kv_send[1] = own v half [s_local, e]
    kv_send = nc.dram_tensor("kv_send", [2, QL, QL], f32r)
    # kv_all[r] = rank r's kv_send within the pair (r == global sequence half)
    kv_all = nc.dram_tensor("kv_all", [2, 2, QL, QL], f32r)

    DT = D // P  # 8 d-tiles
    ET = D // P  # 8 e-tiles
    SB = 512  # s-block for projections
    KB = 512  # key block in attention
    NKB = S // KB  # 4 key blocks

    with tile.TileContext(nc, pool_alloc_mode="queue") as tc, ExitStack() as top:
        cpool = top.enter_context(tc.tile_pool(name="const", bufs=1))
        ident_f = cpool.tile([P, P], f32)
        make_identity(nc, ident_f)
        ident = cpool.tile([P, P], f32r)
        nc.vector.tensor_copy(ident, ident_f)
        ones_f = cpool.tile([P, 2], f32)
        nc.gpsimd.memset(ones_f, 1.0)
        ones = cpool.tile([P, 2], f32r)
        nc.vector.tensor_copy(ones, ones_f)

        # qT stays resident from phase B through phase C: [e, q] layout.
        qT_pool = top.enter_context(tc.tile_pool(name="qT", bufs=1))
        qT = [qT_pool.tile([P, QL], f32r, name=f"qT{e}") for e in range(ET)]

        def load_xn(xn_pool, src_dram, row0):
            xns = []
            for st in range(4):
                xn = xn_pool.tile([P, D], f32r, tag="xn", name=f"xn{st}")
                nc.sync.dma_start(xn, src_dram[row0 + st * P : row0 + (st + 1) * P, :])
                xns.append(xn)
            return xns

        def do_transposes(tpsum, xns, xT_tiles):
            for dt in range(DT):
                for st in range(4):
                    tp = tpsum.tile([P, P], f32r, tag="tp", name="tp")
                    nc.tensor.transpose(tp, xns[st][:, dt * P : (dt + 1) * P], ident)
                    nc.vector.tensor_copy(xT_tiles[dt][:, st * P : (st + 1) * P], tp)

        # ---------------- Phase A: kT / v projection of full x ----------------
        with ExitStack() as pa:
            xn_pool = pa.enter_context(tc.tile_pool(name="xnA", bufs=8))
            xT_pool = pa.enter_context(tc.tile_pool(name="xTA", bufs=2))
            tpsum = pa.enter_context(tc.tile_pool(name="tpsA", bufs=4, space="PSUM"))
            ppool = pa.enter_context(tc.tile_pool(name="ppsA", bufs=4, space="PSUM"))
            stage = pa.enter_context(tc.tile_pool(name="stgA", bufs=6))

            xns_next = load_xn(xn_pool, xk_d, 0)

            wkv_pool = pa.enter_context(tc.tile_pool(name="wkv", bufs=1))
            wk_sb = [wkv_pool.tile([P, D], f32r, name=f"wk{d}") for d in range(DT)]
            wv_sb = [wkv_pool.tile([P, D], f32r, name=f"wv{d}") for d in range(DT)]
            for d in range(DT):
                nc.sync.dma_start(wk_sb[d], wk_d[d * P : (d + 1) * P, :])
            for d in range(DT):
                nc.sync.dma_start(wv_sb[d], wv_d[d * P : (d + 1) * P, :])

            for sb in range(QL // SB):
                xns = xns_next
                if sb + 1 < QL // SB:
                    xns_next = load_xn(xn_pool, xk_d, (sb + 1) * SB)
                xT = [
                    xT_pool.tile([P, SB], f32r, tag=f"xT{dt}", name=f"xT{dt}")
                    for dt in range(DT)
                ]
                do_transposes(tpsum, xns, xT)

                # kT[e, s] = sum_d Wk[d,e]^T x^T[d,s]
                for et in range(ET):
                    pk = ppool.tile([P, SB], f32, tag="pp", name="pk")
                    for dt in range(DT):
                        nc.tensor.matmul(
                            pk,
                            lhsT=wk_sb[dt][:, et * P : (et + 1) * P],
                            rhs=xT[dt],
                            start=(dt == 0),
                            stop=(dt == DT - 1),
                        )
                    ks = stage.tile([P, SB], f32r, tag="stg", name="ks")
                    nc.scalar.copy(ks, pk)
                    nc.sync.dma_start(
                        kv_send[0, et * P : (et + 1) * P, sb * SB : (sb + 1) * SB], ks
                    )
                # v[s, e] = sum_d (x^T[d,s])^T Wv[d,e]
                for st in range(4):
                    for eh in range(2):
                        pv = ppool.tile([P, SB], f32, tag="pp", name="pv")
                        for dt in range(DT):
                            nc.tensor.matmul(
                                pv,
                                lhsT=xT[dt][:, st * P : (st + 1) * P],
                                rhs=wv_sb[dt][:, eh * 512 : (eh + 1) * 512],
                                start=(dt == 0),
                                stop=(dt == DT - 1),
                            )
                        vs = stage.tile([P, SB], f32r, tag="stg", name="vs")
                        nc.scalar.copy(vs, pv)
                        nc.sync.dma_start(
                            kv_send[
                                1,
                                sb * SB + st * P : sb * SB + (st + 1) * P,
                                eh * 512 : (eh + 1) * 512,
                            ],
                            vs,
                        )

        # Exchange K/V halves within each batch pair (ranks 2b, 2b+1).
        nc.gpsimd.collective_compute(
            "AllGather",
            mybir.AluOpType.bypass,
            replica_groups=[[0, 1], [2, 3], [4, 5], [6, 7]],
            ins=[kv_send[:, :, :]],
            outs=[kv_all[:, :, :, :]],
        )

        # ---------------- Phase B: qT projection of own query rows ----------------
        with ExitStack() as pb:
            wq_pool = pb.enter_context(tc.tile_pool(name="wq", bufs=1))
            wq_sb = [wq_pool.tile([P, D], f32r, name=f"wq{d}") for d in range(DT)]
            for d in range(DT):
                nc.sync.dma_start(wq_sb[d], wq_d[d * P : (d + 1) * P, :])

            xn_pool = pb.enter_context(tc.tile_pool(name="xnB", bufs=8))
            xT_pool = pb.enter_context(tc.tile_pool(name="xTB", bufs=2))
            tpsum = pb.enter_context(tc.tile_pool(name="tpsB", bufs=4, space="PSUM"))
            ppool = pb.enter_context(tc.tile_pool(name="ppsB", bufs=4, space="PSUM"))

            xns_next = load_xn(xn_pool, xq_d, 0)
            for sb in range(QL // SB):
                xns = xns_next
                if sb + 1 < QL // SB:
                    xns_next = load_xn(xn_pool, xq_d, (sb + 1) * SB)
                xT = [
                    xT_pool.tile([P, SB], f32r, tag=f"xTq{dt}", name=f"xTq{dt}")
                    for dt in range(DT)
                ]
                do_transposes(tpsum, xns, xT)
                for et in range(ET):
                    pq = ppool.tile([P, SB], f32, tag="ppq", name="pq")
                    for dt in range(DT):
                        nc.tensor.matmul(
                            pq,
                            lhsT=wq_sb[dt][:, et * P : (et + 1) * P],
                            rhs=xT[dt],
                            start=(dt == 0),
                            stop=(dt == DT - 1),
                        )
                    nc.scalar.copy(qT[et][:, sb * SB : (sb + 1) * SB], pq)

        # ---------------- Phase C: attention ----------------
        with ExitStack() as pc:
            mpool = pc.enter_context(tc.tile_pool(name="masks", bufs=1))
            masks_sb = mpool.tile([P, NSLOT * NMASK * GW], f32r)
            for j in range(NSLOT):
                for m in range(NMASK):
                    col = (j * NMASK + m) * GW
                    nc.sync.dma_start(masks_sb[:, col : col + GW], masks_d[j, m])

            acc_pool = pc.enter_context(tc.tile_pool(name="acc", bufs=1))
            # O accumulators: per (slot, q-half) a [128, 1024] fp32 tile.
            O_sb = [
                [acc_pool.tile([P, D], f32, name=f"O{j}_{h}") for h in range(2)]
                for j in range(NSLOT)
            ]
            l_sb = acc_pool.tile([P, 2 * NSLOT], f32)  # row sums, col = 2*slot+half

            kv_pool = pc.enter_context(tc.tile_pool(name="kv", bufs=2))
            w_pool = pc.enter_context(tc.tile_pool(name="wT", bufs=6))
            fin_pool = pc.enter_context(tc.tile_pool(name="fin", bufs=4))
            rec_pool = pc.enter_context(tc.tile_pool(name="recp", bufs=4))
            spsum = pc.enter_context(tc.tile_pool(name="sps", bufs=2, space="PSUM"))
            opsum = pc.enter_context(tc.tile_pool(name="ops", bufs=4, space="PSUM"))
            lpsum = pc.enter_context(tc.tile_pool(name="lps", bufs=2, space="PSUM"))

            for kb in range(NKB):
                kT_sb = [
                    kv_pool.tile([P, KB], f32r, tag=f"kT{e}", name=f"kTs{e}")
                    for e in range(ET)
                ]
                half, loc = kb // 2, kb % 2
                for e in range(ET):
                    nc.sync.dma_start(
                        kT_sb[e],
                        kv_all[half, 0, e * P : (e + 1) * P, loc * KB : (loc + 1) * KB],
                    )
                v_sb = [
                    kv_pool.tile([P, D], f32r, tag=f"v{t}", name=f"vs{t}")
                    for t in range(4)
                ]
                for t in range(4):
                    nc.sync.dma_start(
                        v_sb[t],
                        kv_all[half, 1, loc * KB + t * P : loc * KB + (t + 1) * P, :],
                    )

                for j in range(NSLOT):
                    if kb * 4 >= BOUNDS[j]:
                        continue
                    qcol = j * GW
                    wts = []
                    for kt in range(4):
                        K = kb * 4 + kt  # global key tile
                        sp = spsum.tile([P, GW], f32, tag="sp", name="sp")
                        for e in range(ET):
                            nc.tensor.matmul(
                                sp,
                                lhsT=kT_sb[e][:, kt * P : (kt + 1) * P],
                                rhs=qT[e][:, qcol : qcol + GW],
                                start=(e == 0),
                                stop=(e == ET - 1),
                            )
                        wt = w_pool.tile([P, GW], f32r, tag="wt", name="wt")
                        # w = exp(scores / sqrt(d_k)); scale folded into ACT.
                        nc.scalar.activation(
                            wt, sp, mybir.ActivationFunctionType.Exp, scale=float(SCALE)
                        )
                        m = K - (BOUNDS[j] - NMASK)
                        if m >= 0:
                            mcol = (j * NMASK + m) * GW
                            nc.vector.tensor_mul(
                                wt, wt, masks_sb[:, mcol : mcol + GW]
                            )
                        wts.append(wt)

                    lp = lpsum.tile([P, 4], f32, tag="lp", name="lp")
                    for h in range(2):
                        for eh in range(2):
                            op = opsum.tile([P, 512], f32, tag="op", name="op")
                            for kt in range(4):
                                nc.tensor.matmul(
                                    op,
                                    lhsT=wts[kt][:, h * P : (h + 1) * P],
                                    rhs=v_sb[kt][:, eh * 512 : (eh + 1) * 512],
                                    start=(kt == 0),
                                    stop=(kt == 3),
                                )
                            dst = O_sb[j][h][:, eh * 512 : (eh + 1) * 512]
                            if kb == 0:
                                nc.vector.tensor_copy(dst, op)
                            else:
                                nc.vector.tensor_add(dst, dst, op)
                        for kt in range(4):
                            nc.tensor.matmul(
                                lp[:, 2 * h : 2 * h + 2],
                                lhsT=wts[kt][:, h * P : (h + 1) * P],
                                rhs=ones,
                                start=(kt == 0),
                                stop=(kt == 3),
                            )
                    for h in range(2):
                        lcol = l_sb[:, 2 * j + h : 2 * j + h + 1]
                        if kb == 0:
                            nc.vector.tensor_copy(lcol, lp[:, 2 * h : 2 * h + 1])
                        else:
                            nc.vector.tensor_add(lcol, lcol, lp[:, 2 * h : 2 * h + 1])

                    if kb == BOUNDS[j] // 4 - 1:
                        # last key block for this slot: normalize + store now
                        recip = rec_pool.tile([P, 2], f32, tag="rc", name="recip")
                        nc.vector.reciprocal(recip, l_sb[:, 2 * j : 2 * j + 2])
                        for h in range(2):
                            o = fin_pool.tile([P, D], f32, tag="fo", name="fo")
                            nc.scalar.activation(
                                o,
                                O_sb[j][h],
                                mybir.ActivationFunctionType.Copy,
                                scale=recip[:, h : h + 1],
                            )
                            row = j * GW + h * P
                            nc.sync.dma_start(out_d[row : row + P, :], o)

    nc.compile()
    return nc


def _host_inputs(x, Wq, Wk, Wv):
    in_maps = []
    for c in range(8):
        b, par = c // 2, c % 2
        groups = GROUPS[par]
        rows = np.concatenate(
            [np.arange(GW * g, GW * g + GW) for g in groups]
        )
        xq = np.ascontiguousarray(x[b][rows])
        masks = np.zeros((NSLOT, NMASK, P, GW), np.float32)
        for j, g in enumerate(groups):
            bj = BOUNDS[j]
            for m, kt in enumerate(range(bj - NMASK, bj)):
                kg = P * kt + np.arange(P)[:, None]
                qg = GW * g + np.arange(GW)[None, :]
                masks[j, m] = (kg <= qg).astype(np.float32)
        in_maps.append(
            {
                "xk": np.ascontiguousarray(x[b][par * QL : (par + 1) * QL]),
                "xq": xq,
                "Wq": Wq,
                "Wk": Wk,
                "Wv": Wv,
                "masks": masks,
            }
        )
    return in_maps


def kernel(x, Wq, Wk, Wv):
    from concourse.bass_utils import run_bass_kernel_spmd

    x = np.asarray(x, dtype=np.float32)
    Wq = np.ascontiguousarray(np.asarray(Wq, dtype=np.float32))
    Wk = np.ascontiguousarray(np.asarray(Wk, dtype=np.float32))
    Wv = np.ascontiguousarray(np.asarray(Wv, dtype=np.float32))

    if "nc" not in _cached:
        _cached["nc"] = _build_bass()
    nc = _cached["nc"]

    in_maps = _host_inputs(x, Wq, Wk, Wv)
    res = run_bass_kernel_spmd(nc, in_maps, core_ids=list(range(8)))
    _cached["last_result"] = res

    out = np.zeros((B, S, D), np.float32)
    for c in range(8):
        b, par = c // 2, c % 2
        oc = res.results[c]["out"]
        for j, g in enumerate(GROUPS[par]):
            out[b, GW * g : GW * g + GW] = oc[GW * j : GW * j + GW]
    return out
